# revision 5
# baseline (speedup 1.0000x reference)
"""Trainium2 Bass kernel for nn_MessageFunction (gnn_message_passing).

Computes, per edge e:
    x  = relu(e_vw @ W0.T + b0)                 # [E, 128]
    x  = relu(x @ W1.T + b1)                    # [E, 128]
    eo = (x @ W2.T + b2).reshape(E, 32, 32)     # [E, o, i]
    m  = einsum('eoi,ei->eo', eo, h_w)          # [E, 32]

Sharding: pure edge parallelism across 8 NeuronCores (E/8 = 16384 edges per
core), NNet parameters replicated.

Per-core layout strategy (all host-side pre-transposition, fp16 on-chip
matmul dtypes, fp32 PSUM accumulation):
  - L0/L1 run feature-major (hidden on partitions, edges on the free dim) in
    supertiles of 512 edges; relu+bias evictions on the scalar engine.
  - L2 runs oi-major: 8 chunks of 128 (o,i)-pairs; each chunk is computed for
    the whole 512-edge supertile into one PSUM bank.
  - The per-edge h_w multiply is a fused PSUM-evict+multiply
    (scalar_tensor_tensor) on the vector engine for half the chunks, and a
    scalar-engine copy-evict + 2x-mode tensor_tensor for the other half.
  - The i-contraction is 8 accumulating selection matmuls (0/1 weights) plus
    one small matmul for the b2 term, into PSUM m_newT [32, 512].
"""

import os
import sys
from contextlib import ExitStack

import numpy as np

sys.path.insert(0, "/opt/trn_rl_repo")

import concourse.bass as bass
import concourse.tile as tile
from concourse import bacc, mybir
from concourse._compat import with_exitstack
from concourse.bass_utils import run_bass_kernel_spmd

E = 131072
N_CORES = 8
E_CORE = E // N_CORES          # 16384
TILE_E = 128                   # edges per PE tile (e-major matmul M)
SUPER = 4                      # tiles per supertile
SUPER_E = SUPER * TILE_E       # 512
N_SUPER = E_CORE // SUPER_E    # 32
HID = 128
EF = 16
D = 32                         # D_IN == D_OUT == 32
OI = D * D                     # 1024
N_CHUNK = OI // 128            # 8

F32 = mybir.dt.float32
F16 = mybir.dt.float16

# Split of the 8 oi-chunks between the two evict paths:
# chunks [0, N_STT) -> DVE fused evict+mult; rest -> ACT evict + DVE 2x mult.
N_STT = 4


@with_exitstack
def _edge_mlp_kernel(
    ctx: ExitStack,
    tc: "tile.TileContext",
    out_mT: bass.AP,      # [32, E_CORE] fp32, o-major output
    ev_t: bass.AP,        # [N_SUPER, EF, SUPER_E] fp16  (e_vw transposed)
    hw4: bass.AP,         # [N_SUPER, 128, SUPER_E] fp16 (h_w^T tiled 4x on partitions)
    w0t: bass.AP,         # [EF, HID] fp16
    w1t: bass.AP,         # [HID, HID] fp16
    w2t: bass.AP,         # [HID, OI] fp16
    scm: bass.AP,         # [128, N_CHUNK*32] fp16 selection matrices
    b2rt: bass.AP,        # [D, D] fp16  (b2.reshape(32,32).T)
    b0: bass.AP,          # [HID, 1] fp32
    b1: bass.AP,          # [HID, 1] fp32
):
    nc = tc.nc
    Relu = mybir.ActivationFunctionType.Relu
    Copy = mybir.ActivationFunctionType.Copy

    const = ctx.enter_context(tc.tile_pool(name="const", bufs=1))
    sup = ctx.enter_context(tc.tile_pool(name="sup", bufs=2))
    ypool = ctx.enter_context(tc.tile_pool(name="y", bufs=2))
    opool = ctx.enter_context(tc.tile_pool(name="o", bufs=2))
    ps_x = ctx.enter_context(tc.tile_pool(name="psx", bufs=2, space="PSUM"))
    ps_eo = ctx.enter_context(tc.tile_pool(name="pseo", bufs=3, space="PSUM"))
    ps_m = ctx.enter_context(tc.tile_pool(name="psm", bufs=2, space="PSUM"))

    # --- load constants once ---
    c_w0 = const.tile([EF, HID], F16)
    nc.sync.dma_start(c_w0[:], w0t[:])
    c_w1 = const.tile([HID, HID], F16)
    nc.sync.dma_start(c_w1[:], w1t[:])
    c_w2 = const.tile([HID, OI], F16)
    nc.sync.dma_start(c_w2[:], w2t[:])
    c_sc = const.tile([128, N_CHUNK * D], F16)
    nc.sync.dma_start(c_sc[:], scm[:])
    c_b2 = const.tile([D, D], F16)
    nc.sync.dma_start(c_b2[:], b2rt[:])
    c_b0 = const.tile([HID, 1], F32)
    nc.sync.dma_start(c_b0[:], b0[:])
    c_b1 = const.tile([HID, 1], F32)
    nc.sync.dma_start(c_b1[:], b1[:])

    for s in range(N_SUPER):
        ev = sup.tile([EF, SUPER_E], F16, tag="ev")
        nc.sync.dma_start(ev[:], ev_t[s])
        hw = sup.tile([128, SUPER_E], F16, tag="hw")
        nc.sync.dma_start(hw[:], hw4[s])

        # L0: x1T[h, e] = sum_f W0T[f, h] * evT[f, e]
        x1p = ps_x.tile([HID, SUPER_E], F32, tag="xp")
        nc.tensor.matmul(x1p[:], c_w0[:], ev[:])
        x1s = sup.tile([HID, SUPER_E], F16, tag="x1s")
        nc.scalar.activation(x1s[:], x1p[:], Relu, bias=c_b0[:])

        # L1: x2T[h2, e] = sum_h W1T[h, h2] * x1T[h, e]
        x2p = ps_x.tile([HID, SUPER_E], F32, tag="xp")
        nc.tensor.matmul(x2p[:], c_w1[:], x1s[:])
        x2s = sup.tile([HID, SUPER_E], F16, tag="x2s")
        nc.scalar.activation(x2s[:], x2p[:], Relu, bias=c_b1[:])

        # b2 term: m_newT[o, e] = sum_i b2r[o, i] * hwT[i, e]  (accum start)
        mp = ps_m.tile([D, SUPER_E], F32, tag="mp")
        nc.tensor.matmul(
            mp[:], c_b2[:], hw[0:D, :], start=True, stop=False
        )

        # L2 + h_w multiply, chunk by chunk (chunk = 128 (o,i)-pairs)
        for c in range(N_CHUNK):
            eo = ps_eo.tile([128, SUPER_E], F32, tag="eo")
            for t in range(SUPER):
                nc.tensor.matmul(
                    eo[:, t * TILE_E : (t + 1) * TILE_E],
                    c_w2[:, c * 128 : (c + 1) * 128],
                    x2s[:, t * TILE_E : (t + 1) * TILE_E],
                )
            yc = ypool.tile([128, SUPER_E], F16, tag=f"y{c}")
            if c < N_STT:
                # fused evict+mult on DVE: y = eo * hw
                nc.vector.scalar_tensor_tensor(
                    yc[:], eo[:], 1.0, hw[:],
                    op0=mybir.AluOpType.mult, op1=mybir.AluOpType.mult,
                )
            else:
                # ACT evicts (fp32 psum -> fp16 sbuf), DVE multiplies at 2x
                eos = ypool.tile([128, SUPER_E], F16, tag=f"eos{c % 2}")
                nc.scalar.activation(eos[:], eo[:], Copy)
                nc.vector.tensor_mul(yc[:], eos[:], hw[:])
            # i-contraction: accumulate S_c.T @ y_c into m_newT
            nc.tensor.matmul(
                mp[:], c_sc[:, c * D : (c + 1) * D], yc[:],
                start=False, stop=(c == N_CHUNK - 1),
            )

        # evict m_newT and store
        ms = opool.tile([D, SUPER_E], F32, tag="ms")
        nc.scalar.activation(ms[:], mp[:], Copy)
        nc.sync.dma_start(out_mT[:, s * SUPER_E : (s + 1) * SUPER_E], ms[:])


def _build_bass():
    nc = bacc.Bacc("TRN2", target_bir_lowering=False, debug=False)
    d = {}
    d["ev_t"] = nc.dram_tensor("ev_t", [N_SUPER, EF, SUPER_E], F16, kind="ExternalInput")
    d["hw4"] = nc.dram_tensor("hw4", [N_SUPER, 128, SUPER_E], F16, kind="ExternalInput")
    d["w0t"] = nc.dram_tensor("w0t", [EF, HID], F16, kind="ExternalInput")
    d["w1t"] = nc.dram_tensor("w1t", [HID, HID], F16, kind="ExternalInput")
    d["w2t"] = nc.dram_tensor("w2t", [HID, OI], F16, kind="ExternalInput")
    d["scm"] = nc.dram_tensor("scm", [128, N_CHUNK * D], F16, kind="ExternalInput")
    d["b2rt"] = nc.dram_tensor("b2rt", [D, D], F16, kind="ExternalInput")
    d["b0"] = nc.dram_tensor("b0", [HID, 1], F32, kind="ExternalInput")
    d["b1"] = nc.dram_tensor("b1", [HID, 1], F32, kind="ExternalInput")
    out = nc.dram_tensor("out_mT", [D, E_CORE], F32, kind="ExternalOutput")

    with tile.TileContext(nc) as tc:
        _edge_mlp_kernel(
            tc,
            out.ap(),
            d["ev_t"].ap(), d["hw4"].ap(),
            d["w0t"].ap(), d["w1t"].ap(), d["w2t"].ap(),
            d["scm"].ap(), d["b2rt"].ap(),
            d["b0"].ap(), d["b1"].ap(),
        )
    nc.compile()
    return nc


def _prep_host_inputs(h_w, e_vw, W0, b0, W1, b1, W2, b2):
    """Build per-core input maps (all numpy, cheap)."""
    # shared (replicated) parameters
    w0t = np.ascontiguousarray(W0.T).astype(np.float16)            # [16, 128]
    w1t = np.ascontiguousarray(W1.T).astype(np.float16)            # [128, 128]
    w2t = np.ascontiguousarray(W2.T).astype(np.float16)            # [128, 1024]
    b2r = b2.reshape(D, D)                                          # [o, i]
    b2rt = np.ascontiguousarray(b2r.T).astype(np.float16)           # [i, o]
    b0c = np.ascontiguousarray(b0.reshape(HID, 1)).astype(np.float32)
    b1c = np.ascontiguousarray(b1.reshape(HID, 1)).astype(np.float32)
    # selection matrices: scm[p, c*32 + o] = 1 iff o == 4c + p//32
    scm = np.zeros((128, N_CHUNK * D), np.float16)
    p = np.arange(128)
    for c in range(N_CHUNK):
        scm[p, c * D + 4 * c + p // D] = 1.0

    in_maps = []
    for core in range(N_CORES):
        sl = slice(core * E_CORE, (core + 1) * E_CORE)
        ev_c = e_vw[sl]                                             # [16384, 16]
        hw_c = h_w[sl]                                              # [16384, 32]
        # ev_t[s, f, t*128+e] = ev_c[s*512 + t*128 + e, f]
        ev_t = np.ascontiguousarray(
            ev_c.reshape(N_SUPER, SUPER_E, EF).transpose(0, 2, 1)
        ).astype(np.float16)
        # hwT_s[s, i, e] then tiled 4x on partition axis
        hw_t = hw_c.reshape(N_SUPER, SUPER_E, D).transpose(0, 2, 1)  # [32, 32, 512]
        hw4 = np.ascontiguousarray(np.tile(hw_t, (1, 4, 1))).astype(np.float16)
        in_maps.append({
            "ev_t": ev_t, "hw4": hw4,
            "w0t": w0t, "w1t": w1t, "w2t": w2t,
            "scm": scm, "b2rt": b2rt, "b0": b0c, "b1": b1c,
        })
    return in_maps


_CACHE = {}


def kernel(h_v, h_w, e_vw, W0, b0, W1, b1, W2, b2, _trace=False, _results=None):
    # h_v is unused by the reference computation (only its trailing dim of 1
    # matters there); the message depends on h_w, e_vw and the NNet params.
    del h_v
    in_maps = _prep_host_inputs(
        np.asarray(h_w, np.float32), np.asarray(e_vw, np.float32),
        np.asarray(W0, np.float32), np.asarray(b0, np.float32),
        np.asarray(W1, np.float32), np.asarray(b1, np.float32),
        np.asarray(W2, np.float32), np.asarray(b2, np.float32),
    )
    if "nc" not in _CACHE:
        _CACHE["nc"] = _build_bass()
    nc = _CACHE["nc"]
    res = run_bass_kernel_spmd(
        nc, in_maps, core_ids=list(range(N_CORES)), trace=_trace,
    )
    if _results is not None:
        _results.append(res)
    parts = [res.results[c]["out_mT"] for c in range(N_CORES)]
    full_T = np.concatenate(parts, axis=1)          # [32, E]
    return np.ascontiguousarray(full_T.T)           # [E, 32]


if __name__ == "__main__":
    import reference
    inputs = reference.setup_inputs()
    inputs = {k: np.asarray(v) for k, v in inputs.items()}
    expected = np.asarray(reference.reference(**inputs))
    actual = kernel(**inputs)
    err = np.abs(actual - expected)
    denom = np.abs(expected).max()
    print("max abs err:", err.max(), "rel err:", err.max() / denom)


# revision 8
# speedup vs baseline: 1.1068x; 1.1068x over previous
"""Trainium2 Bass kernel for nn_MessageFunction (gnn_message_passing).

Computes, per edge e:
    x  = relu(e_vw @ W0.T + b0)                 # [E, 128]
    x  = relu(x @ W1.T + b1)                    # [E, 128]
    eo = (x @ W2.T + b2).reshape(E, 32, 32)     # [E, o, i]
    m  = einsum('eoi,ei->eo', eo, h_w)          # [E, 32]

Sharding: pure edge parallelism across 8 NeuronCores (E/8 = 16384 edges per
core), NNet parameters replicated.

Per-core layout strategy (all host-side pre-transposition, fp16 on-chip
matmul dtypes, fp32 PSUM accumulation):
  - L0/L1 run feature-major (hidden on partitions, edges on the free dim) in
    supertiles of 512 edges; relu+bias evictions on the scalar engine.
  - L2 runs oi-major: 8 chunks of 128 (o,i)-pairs; each chunk is computed for
    the whole 512-edge supertile into one PSUM bank.
  - The per-edge h_w multiply is a fused PSUM-evict+multiply
    (scalar_tensor_tensor) on the vector engine for half the chunks, and a
    scalar-engine copy-evict + 2x-mode tensor_tensor for the other half.
  - The i-contraction is 8 accumulating selection matmuls (0/1 weights) plus
    one small matmul for the b2 term, into PSUM m_newT [32, 512].
"""

import os
import sys
from contextlib import ExitStack

import numpy as np

sys.path.insert(0, "/opt/trn_rl_repo")

import concourse.bass as bass
import concourse.tile as tile
from concourse import bacc, mybir
from concourse._compat import with_exitstack
from concourse.bass_utils import run_bass_kernel_spmd

E = 131072
N_CORES = 8
E_CORE = E // N_CORES          # 16384
TILE_E = 128                   # edges per PE tile (e-major matmul M)
SUPER = 4                      # tiles per supertile
SUPER_E = SUPER * TILE_E       # 512
N_SUPER = E_CORE // SUPER_E    # 32
HID = 128
EF = 16
D = 32                         # D_IN == D_OUT == 32
OI = D * D                     # 1024
N_CHUNK = OI // 128            # 8

F32 = mybir.dt.float32
F16 = mybir.dt.float16

# Split of the 8 oi-chunks between the two evict paths:
# chunks [0, N_STT) -> DVE fused evict+mult; rest -> ACT evict + DVE 2x mult.
N_STT = 4


@with_exitstack
def _edge_mlp_kernel(
    ctx: ExitStack,
    tc: "tile.TileContext",
    out_mT: bass.AP,      # [32, E_CORE] fp32, o-major output
    ev_t: bass.AP,        # [N_SUPER, EF, SUPER_E] fp16  (e_vw transposed)
    hw4: bass.AP,         # [N_SUPER, 128, SUPER_E] fp16 (h_w^T tiled 4x on partitions)
    w0t: bass.AP,         # [EF, HID] fp16
    w1t: bass.AP,         # [HID, HID] fp16
    w2t: bass.AP,         # [HID, OI] fp16
    scm: bass.AP,         # [128, N_CHUNK*32] fp16 selection matrices
    b2rt: bass.AP,        # [D, D] fp16  (b2.reshape(32,32).T)
    b0: bass.AP,          # [HID, 1] fp32
    b1: bass.AP,          # [HID, 1] fp32
):
    nc = tc.nc
    Relu = mybir.ActivationFunctionType.Relu
    Copy = mybir.ActivationFunctionType.Copy

    const = ctx.enter_context(tc.tile_pool(name="const", bufs=1))
    sup = ctx.enter_context(tc.tile_pool(name="sup", bufs=2))
    ypool = ctx.enter_context(tc.tile_pool(name="y", bufs=2))
    opool = ctx.enter_context(tc.tile_pool(name="o", bufs=2))
    ps_x = ctx.enter_context(tc.tile_pool(name="psx", bufs=2, space="PSUM"))
    ps_eo = ctx.enter_context(tc.tile_pool(name="pseo", bufs=2, space="PSUM"))
    ps_m = ctx.enter_context(tc.tile_pool(name="psm", bufs=2, space="PSUM"))

    # --- load constants once ---
    c_w0 = const.tile([EF, HID], F16)
    nc.sync.dma_start(c_w0[:], w0t[:])
    c_w1 = const.tile([HID, HID], F16)
    nc.sync.dma_start(c_w1[:], w1t[:])
    c_w2 = const.tile([HID, OI], F16)
    nc.sync.dma_start(c_w2[:], w2t[:])
    c_sc = const.tile([128, N_CHUNK * D], F16)
    nc.sync.dma_start(c_sc[:], scm[:])
    c_b2 = const.tile([D, D], F16)
    nc.sync.dma_start(c_b2[:], b2rt[:])
    c_b0 = const.tile([HID, 1], F32)
    nc.sync.dma_start(c_b0[:], b0[:])
    c_b1 = const.tile([HID, 1], F32)
    nc.sync.dma_start(c_b1[:], b1[:])

    for s in range(N_SUPER):
        ev = sup.tile([EF, SUPER_E], F16, tag="ev")
        nc.sync.dma_start(ev[:], ev_t[s])
        hw = sup.tile([128, SUPER_E], F16, tag="hw")
        nc.sync.dma_start(hw[:], hw4[s])

        # L0: x1T[h, e] = sum_f W0T[f, h] * evT[f, e]
        x1p = ps_x.tile([HID, SUPER_E], F32, tag="xp")
        nc.tensor.matmul(x1p[:], c_w0[:], ev[:])
        x1s = sup.tile([HID, SUPER_E], F16, tag="x1s")
        nc.scalar.activation(x1s[:], x1p[:], Relu, bias=c_b0[:])

        # L1: x2T[h2, e] = sum_h W1T[h, h2] * x1T[h, e]
        x2p = ps_x.tile([HID, SUPER_E], F32, tag="xp")
        nc.tensor.matmul(x2p[:], c_w1[:], x1s[:])
        x2s = sup.tile([HID, SUPER_E], F16, tag="x2s")
        nc.scalar.activation(x2s[:], x2p[:], Relu, bias=c_b1[:])

        # b2 term: m_newT[o, e] = sum_i b2r[o, i] * hwT[i, e]  (accum start)
        mp = ps_m.tile([D, SUPER_E], F32, tag="mp")
        nc.tensor.matmul(
            mp[:], c_b2[:], hw[0:D, :], start=True, stop=False
        )

        # L2 + h_w multiply, processed as chunk pairs (chunk = 128
        # (o,i)-pairs for the whole 512-edge supertile = one PSUM bank;
        # a pair = 2 banks so evict ops run at FD=1024)
        hw2 = hw[:].rearrange("p (x e) -> p x e", x=1).broadcast_to(
            [128, 2, SUPER_E]
        )  # hw broadcast over the chunk pair (step-0 middle dim)
        for p in range(N_CHUNK // 2):
            c0 = 2 * p
            eo = ps_eo.tile([128, 2, SUPER_E], F32, tag="eo")
            nc.tensor.matmul(eo[:, 0, :], c_w2[:, c0 * 128 : (c0 + 1) * 128], x2s[:])
            nc.tensor.matmul(eo[:, 1, :], c_w2[:, (c0 + 1) * 128 : (c0 + 2) * 128], x2s[:])
            yc = ypool.tile([128, 2, SUPER_E], F16, tag=f"y{p}")
            if p < N_STT // 2:
                # fused evict+mult on DVE: y = eo * hw (hw broadcast over pair)
                nc.vector.scalar_tensor_tensor(
                    yc[:], eo[:], 1.0, hw2[:],
                    op0=mybir.AluOpType.mult, op1=mybir.AluOpType.mult,
                )
            else:
                # ACT evicts (fp32 psum -> fp16 sbuf), DVE multiplies at 2x
                eos = ypool.tile([128, 2, SUPER_E], F16, tag=f"eos{p % 2}")
                nc.scalar.activation(eos[:], eo[:], Copy)
                nc.vector.tensor_mul(yc[:], eos[:], hw2[:])
            # i-contraction: accumulate S_c.T @ y_c into m_newT
            for j in range(2):
                c = c0 + j
                nc.tensor.matmul(
                    mp[:], c_sc[:, c * D : (c + 1) * D], yc[:, j, :],
                    start=False, stop=(c == N_CHUNK - 1),
                )

        # evict m_newT and store
        ms = opool.tile([D, SUPER_E], F32, tag="ms")
        nc.scalar.activation(ms[:], mp[:], Copy)
        nc.sync.dma_start(out_mT[:, s * SUPER_E : (s + 1) * SUPER_E], ms[:])


def _build_bass():
    nc = bacc.Bacc("TRN2", target_bir_lowering=False, debug=False)
    d = {}
    d["ev_t"] = nc.dram_tensor("ev_t", [N_SUPER, EF, SUPER_E], F16, kind="ExternalInput")
    d["hw4"] = nc.dram_tensor("hw4", [N_SUPER, 128, SUPER_E], F16, kind="ExternalInput")
    d["w0t"] = nc.dram_tensor("w0t", [EF, HID], F16, kind="ExternalInput")
    d["w1t"] = nc.dram_tensor("w1t", [HID, HID], F16, kind="ExternalInput")
    d["w2t"] = nc.dram_tensor("w2t", [HID, OI], F16, kind="ExternalInput")
    d["scm"] = nc.dram_tensor("scm", [128, N_CHUNK * D], F16, kind="ExternalInput")
    d["b2rt"] = nc.dram_tensor("b2rt", [D, D], F16, kind="ExternalInput")
    d["b0"] = nc.dram_tensor("b0", [HID, 1], F32, kind="ExternalInput")
    d["b1"] = nc.dram_tensor("b1", [HID, 1], F32, kind="ExternalInput")
    out = nc.dram_tensor("out_mT", [D, E_CORE], F32, kind="ExternalOutput")

    with tile.TileContext(nc) as tc:
        _edge_mlp_kernel(
            tc,
            out.ap(),
            d["ev_t"].ap(), d["hw4"].ap(),
            d["w0t"].ap(), d["w1t"].ap(), d["w2t"].ap(),
            d["scm"].ap(), d["b2rt"].ap(),
            d["b0"].ap(), d["b1"].ap(),
        )
    nc.compile()
    return nc


def _prep_host_inputs(h_w, e_vw, W0, b0, W1, b1, W2, b2):
    """Build per-core input maps (all numpy, cheap)."""
    # shared (replicated) parameters
    w0t = np.ascontiguousarray(W0.T).astype(np.float16)            # [16, 128]
    w1t = np.ascontiguousarray(W1.T).astype(np.float16)            # [128, 128]
    w2t = np.ascontiguousarray(W2.T).astype(np.float16)            # [128, 1024]
    b2r = b2.reshape(D, D)                                          # [o, i]
    b2rt = np.ascontiguousarray(b2r.T).astype(np.float16)           # [i, o]
    b0c = np.ascontiguousarray(b0.reshape(HID, 1)).astype(np.float32)
    b1c = np.ascontiguousarray(b1.reshape(HID, 1)).astype(np.float32)
    # selection matrices: scm[p, c*32 + o] = 1 iff o == 4c + p//32
    scm = np.zeros((128, N_CHUNK * D), np.float16)
    p = np.arange(128)
    for c in range(N_CHUNK):
        scm[p, c * D + 4 * c + p // D] = 1.0

    in_maps = []
    for core in range(N_CORES):
        sl = slice(core * E_CORE, (core + 1) * E_CORE)
        ev_c = e_vw[sl]                                             # [16384, 16]
        hw_c = h_w[sl]                                              # [16384, 32]
        # ev_t[s, f, t*128+e] = ev_c[s*512 + t*128 + e, f]
        ev_t = np.ascontiguousarray(
            ev_c.reshape(N_SUPER, SUPER_E, EF).transpose(0, 2, 1)
        ).astype(np.float16)
        # hwT_s[s, i, e] then tiled 4x on partition axis
        hw_t = hw_c.reshape(N_SUPER, SUPER_E, D).transpose(0, 2, 1)  # [32, 32, 512]
        hw4 = np.ascontiguousarray(np.tile(hw_t, (1, 4, 1))).astype(np.float16)
        in_maps.append({
            "ev_t": ev_t, "hw4": hw4,
            "w0t": w0t, "w1t": w1t, "w2t": w2t,
            "scm": scm, "b2rt": b2rt, "b0": b0c, "b1": b1c,
        })
    return in_maps


_CACHE = {}


def kernel(h_v, h_w, e_vw, W0, b0, W1, b1, W2, b2, _trace=False, _results=None):
    # h_v is unused by the reference computation (only its trailing dim of 1
    # matters there); the message depends on h_w, e_vw and the NNet params.
    del h_v
    in_maps = _prep_host_inputs(
        np.asarray(h_w, np.float32), np.asarray(e_vw, np.float32),
        np.asarray(W0, np.float32), np.asarray(b0, np.float32),
        np.asarray(W1, np.float32), np.asarray(b1, np.float32),
        np.asarray(W2, np.float32), np.asarray(b2, np.float32),
    )
    if "nc" not in _CACHE:
        _CACHE["nc"] = _build_bass()
    nc = _CACHE["nc"]
    res = run_bass_kernel_spmd(
        nc, in_maps, core_ids=list(range(N_CORES)), trace=_trace,
    )
    if _results is not None:
        _results.append(res)
    parts = [res.results[c]["out_mT"] for c in range(N_CORES)]
    full_T = np.concatenate(parts, axis=1)          # [32, E]
    return np.ascontiguousarray(full_T.T)           # [E, 32]


if __name__ == "__main__":
    import reference
    inputs = reference.setup_inputs()
    inputs = {k: np.asarray(v) for k, v in inputs.items()}
    expected = np.asarray(reference.reference(**inputs))
    actual = kernel(**inputs)
    err = np.abs(actual - expected)
    denom = np.abs(expected).max()
    print("max abs err:", err.max(), "rel err:", err.max() / denom)
